# Initial kernel scaffold
#
"""Trainium2 Bass kernel for the CriterionG segment-reduce loss.

Computes, for close_er [N, C], y [N], max_dis [C], margin scalar:
    ce[n]  = close_er[n, y[n]]
    z[n]   = ce[n] - max_dis[y[n]] - margin
    nll[n] = -log(clamp(sigmoid(z), EPS, 1-EPS)) == softplus(-z) for |z| < 16
    per-class mean of nll over samples with y == c, averaged over non-empty
    classes.

Strategy: data-parallel over N across 8 NeuronCores.  Each core streams its
[32768, 512] slice of close_er (cast fp32->fp16 during DMA), and per
[128, 512] tile computes
    V = T - (max_dis + margin)          (VectorE tensor_tensor)
    H = (iota == y_col)                 (VectorE tensor_scalar, one-hot)
    z = sum(V * H)                      (VectorE tensor_tensor_reduce)
    nll = Softplus(-z)                  (ScalarE)
    psum[2, 512] += [nll, 1]^T @ H      (TensorE, per-class sums + counts)
Host sums the 8 [2, 512] partials and finishes the tiny per-class mean /
class-average arithmetic.
"""

import numpy as np

N, C = 262144, 512
NCORES = 8
P = 128
NPC = N // NCORES        # rows per core = 32768
J = NPC // P             # tiles per core = 256
JG = 8                   # tiles per DMA group (2 MB fp32 per group)

_program_cache = {}


def _build_program(jtiles, jgroup):
    import concourse.bass as bass
    import concourse.mybir as mybir
    import concourse.tile as tile

    f16 = mybir.dt.float16
    f32 = mybir.dt.float32
    alu = mybir.AluOpType

    nc = bass.Bass()
    ce = nc.declare_dram_parameter("ce", [P, jtiles, C], f32, isOutput=False)
    ysb = nc.declare_dram_parameter("ysb", [P, jtiles], f32, isOutput=False)
    iota = nc.declare_dram_parameter("iota", [P, C], f16, isOutput=False)
    dmd = nc.declare_dram_parameter("dmd", [P, C], f16, isOutput=False)
    partial = nc.declare_dram_parameter("partial", [2, C], f32, isOutput=True)

    ngroups = jtiles // jgroup
    assert ngroups * jgroup == jtiles

    with tile.TileContext(nc) as tc:
        with (
            tc.tile_pool(name="const", bufs=1) as constp,
            tc.tile_pool(name="big", bufs=3) as bigp,
            tc.tile_pool(name="h", bufs=2 * jgroup + 2) as hp,
            tc.tile_pool(name="work", bufs=3) as workp,
            tc.tile_pool(name="zg", bufs=4) as zgp,
            tc.tile_pool(name="psum", bufs=1, space="PSUM") as psump,
        ):
            iota_t = constp.tile([P, C], f16, tag="iota")
            nc.sync.dma_start(out=iota_t[:], in_=iota[:])
            dmd_t = constp.tile([P, C], f16, tag="dmd")
            nc.sync.dma_start(out=dmd_t[:], in_=dmd[:])
            ysb_t = constp.tile([P, jtiles], f32, tag="ysb")
            nc.sync.dma_start(out=ysb_t[:], in_=ysb[:])

            ps = psump.tile([2, C], f32)

            for g in range(ngroups):
                big = bigp.tile([P, jgroup * C], f16, tag="big")
                nc.gpsimd.dma_start(out=big[:], in_=ce[:, g * jgroup:(g + 1) * jgroup, :])

                z_g = zgp.tile([P, jgroup], f32, tag="zg")
                nllo = zgp.tile([P, jgroup, 2], f16, tag="nllo")
                nc.vector.memset(nllo[:], 1.0)

                hs = []
                for jl in range(jgroup):
                    j = g * jgroup + jl
                    T = big[:, jl * C:(jl + 1) * C]
                    V = workp.tile([P, C], f16, tag="V")
                    nc.vector.tensor_tensor(out=V[:], in0=T, in1=dmd_t[:], op=alu.subtract)
                    H = hp.tile([P, C], f16, tag="H")
                    nc.vector.tensor_scalar(
                        out=H[:], in0=iota_t[:],
                        scalar1=ysb_t[:, j:j + 1], scalar2=None,
                        op0=alu.is_equal,
                    )
                    hs.append(H)
                    scr = workp.tile([P, C], f16, tag="scr")
                    nc.vector.tensor_tensor_reduce(
                        out=scr[:], in0=V[:], in1=H[:],
                        scale=1.0, scalar=0.0,
                        op0=alu.mult, op1=alu.add,
                        accum_out=z_g[:, jl:jl + 1],
                    )

                # nll = softplus(-z) for the whole group in one ScalarE op,
                # written strided into the [nll | 1] matmul operand.
                nc.scalar.activation(
                    out=nllo[:, :, 0], in_=z_g[:],
                    func=mybir.ActivationFunctionType.Softplus,
                    scale=-1.0,
                )

                for jl in range(jgroup):
                    j = g * jgroup + jl
                    nc.tensor.matmul(
                        out=ps[:], lhsT=nllo[:, jl, :], rhs=hs[jl][:],
                        start=(j == 0), stop=(j == jtiles - 1),
                    )

            out_sb = constp.tile([2, C], f32, tag="out")
            nc.vector.tensor_copy(out=out_sb[:], in_=ps[:])
            nc.sync.dma_start(out=partial[:], in_=out_sb[:])

    return nc


def _get_program(jtiles=J, jgroup=JG):
    key = (jtiles, jgroup)
    if key not in _program_cache:
        _program_cache[key] = _build_program(jtiles, jgroup)
    return _program_cache[key]


def _make_in_maps(close_er, y, max_dis, margin, ncores=NCORES, jtiles=J):
    close_er = np.ascontiguousarray(np.asarray(close_er, dtype=np.float32))
    y = np.asarray(y)
    max_dis = np.asarray(max_dis, dtype=np.float32)
    margin = np.float32(np.asarray(margin))

    npc = P * jtiles
    iota_np = np.ascontiguousarray(
        np.broadcast_to(np.arange(C, dtype=np.float16), (P, C))
    )
    dmd_np = np.ascontiguousarray(
        np.broadcast_to((max_dis + margin).astype(np.float16), (P, C))
    )
    in_maps = []
    for c in range(ncores):
        sl = slice(c * npc, (c + 1) * npc)
        in_maps.append({
            "ce": close_er[sl].reshape(P, jtiles, C),
            "ysb": np.ascontiguousarray(
                y[sl].reshape(P, jtiles).astype(np.float32)
            ),
            "iota": iota_np,
            "dmd": dmd_np,
        })
    return in_maps


def _finish(partials):
    """partials: [ncores, 2, C] -> final scalar, replicating reference math."""
    partials = np.asarray(partials, dtype=np.float64)
    sums = partials[:, 0, :].sum(axis=0)
    counts = partials[:, 1, :].sum(axis=0)
    nonempty = counts > 0
    means = np.where(nonempty, sums / np.maximum(counts, 1.0), 0.0)
    jn = nonempty.sum()
    return np.asarray(means.sum() / jn, dtype=np.float32)


def kernel(close_er, y, max_dis, margin):
    from concourse.bass_utils import run_bass_kernel_spmd

    nc = _get_program()
    in_maps = _make_in_maps(close_er, y, max_dis, margin)
    res = run_bass_kernel_spmd(nc, in_maps, list(range(NCORES)))
    partials = np.stack([res.results[i]["partial"] for i in range(NCORES)])
    return _finish(partials)


if __name__ == "__main__":
    rng = np.random.default_rng(0)
    close_er = rng.standard_normal((N, C), dtype=np.float32)
    y = rng.integers(0, C, size=N).astype(np.int32)
    max_dis = rng.standard_normal(C).astype(np.float32)
    margin = np.float32(0.5)
    out = kernel(close_er, y, max_dis, margin)
    print("kernel output:", out)


# revision 14
# speedup vs baseline: 1.3104x; 1.3104x over previous
"""Trainium2 Bass kernel for the CriterionG segment-reduce loss.

Computes, for close_er [N, C], y [N], max_dis [C], margin scalar:
    ce[n]  = close_er[n, y[n]]
    z[n]   = ce[n] - max_dis[y[n]] - margin
    nll[n] = -log(clamp(sigmoid(z), EPS, 1-EPS)) == softplus(-z) for |z| < 16
    per-class mean of nll over samples with y == c, averaged over non-empty
    classes.

Strategy: data-parallel over N across 8 NeuronCores.  Each core streams its
[32768, 512] slice of close_er (cast fp32->fp16 during DMA), and per
[128, 512] tile computes
    V = T - (max_dis + margin)          (VectorE tensor_tensor)
    H = (iota == y_col)                 (VectorE tensor_scalar, one-hot)
    z = sum(V * H)                      (VectorE tensor_tensor_reduce)
    nll = Softplus(-z)                  (ScalarE)
    psum[2, 512] += [nll, 1]^T @ H      (TensorE, per-class sums + counts)
Host sums the 8 [2, 512] partials and finishes the tiny per-class mean /
class-average arithmetic.
"""

import numpy as np

N, C = 262144, 512
NCORES = 8
P = 128
NPC = N // NCORES        # rows per core = 32768
J = NPC // P             # tiles per core = 256
JG = 8                   # tiles per DMA group (2 MB fp32 per group)

_program_cache = {}


def _build_program(jtiles, jgroup):
    import concourse.bacc as bacc
    import concourse.mybir as mybir
    import concourse.tile as tile

    f16 = mybir.dt.float16
    f32 = mybir.dt.float32
    alu = mybir.AluOpType

    # Bacc (not bass.Bass): its finalize() runs the TRN2 hardware-constraint
    # passes — sync-wait splitting (max 1 wait/instruction), ISA subclass
    # conversion, ACT table loads.  Raw Bass programs die in walrus codegen.
    nc = bacc.Bacc()
    ce = nc.declare_dram_parameter("ce", [P, jtiles, C], f16, isOutput=False)
    ysb = nc.declare_dram_parameter("ysb", [P, jtiles], f32, isOutput=False)
    iota = nc.declare_dram_parameter("iota", [P, C], f16, isOutput=False)
    dmd = nc.declare_dram_parameter("dmd", [P, C], f16, isOutput=False)
    partial = nc.declare_dram_parameter("partial", [2, C], f32, isOutput=True)

    ngroups = jtiles // jgroup
    assert ngroups * jgroup == jtiles

    with tile.TileContext(nc) as tc:
        with (
            tc.tile_pool(name="const", bufs=1) as constp,
            tc.tile_pool(name="big", bufs=3) as bigp,
            tc.tile_pool(name="h", bufs=2 * jgroup + 2) as hp,
            tc.tile_pool(name="work", bufs=3) as workp,
            tc.tile_pool(name="zg", bufs=4) as zgp,
            tc.tile_pool(name="psum", bufs=1, space="PSUM") as psump,
        ):
            iota_t = constp.tile([P, C], f16, tag="iota")
            nc.sync.dma_start(out=iota_t[:], in_=iota[:])
            dmd_t = constp.tile([P, C], f16, tag="dmd")
            nc.sync.dma_start(out=dmd_t[:], in_=dmd[:])
            ysb_t = constp.tile([P, jtiles], f32, tag="ysb")
            nc.sync.dma_start(out=ysb_t[:], in_=ysb[:])

            ps = psump.tile([2, C], f32)

            for g in range(ngroups):
                big = bigp.tile([P, jgroup * C], f16, tag="big")
                nc.gpsimd.dma_start(out=big[:], in_=ce[:, g * jgroup:(g + 1) * jgroup, :])

                z_g = zgp.tile([P, jgroup], f32, tag="zg")
                nllo = zgp.tile([P, jgroup, 2], f16, tag="nllo")
                nc.vector.memset(nllo[:], 1.0)

                hs = []
                for jl in range(jgroup):
                    j = g * jgroup + jl
                    T = big[:, jl * C:(jl + 1) * C]
                    # Engine split: PoolE builds the one-hot and the masked
                    # product, VectorE does the subtract and the reduction —
                    # two big ops each per tile instead of four on VectorE.
                    V = workp.tile([P, C], f16, tag="V")
                    nc.vector.tensor_tensor(out=V[:], in0=T, in1=dmd_t[:], op=alu.subtract)
                    H = hp.tile([P, C], f16, tag="H")
                    nc.gpsimd.tensor_scalar(
                        out=H[:], in0=iota_t[:],
                        scalar1=ysb_t[:, j:j + 1], scalar2=None,
                        op0=alu.is_equal,
                    )
                    hs.append(H)
                    scr = workp.tile([P, C], f16, tag="scr")
                    nc.gpsimd.tensor_tensor(
                        out=scr[:], in0=V[:], in1=H[:], op=alu.mult,
                    )
                    nc.vector.tensor_reduce(
                        out=z_g[:, jl:jl + 1], in_=scr[:],
                        axis=mybir.AxisListType.X, op=alu.add,
                    )

                # nll = softplus(-z) = log(1 + exp(-z)), written strided into
                # the [nll | 1] matmul operand.  Exp and Ln share one ACT
                # table set (natural_log_exp_and_others).
                e_g = zgp.tile([P, jgroup], f32, tag="eg")
                nc.scalar.activation(
                    out=e_g[:], in_=z_g[:],
                    func=mybir.ActivationFunctionType.Exp,
                    scale=-1.0,
                )
                nc.scalar.activation(
                    out=nllo[:, :, 0], in_=e_g[:],
                    func=mybir.ActivationFunctionType.Ln,
                    bias=1.0,
                )

                for jl in range(jgroup):
                    j = g * jgroup + jl
                    nc.tensor.matmul(
                        out=ps[:], lhsT=nllo[:, jl, :], rhs=hs[jl][:],
                        start=(j == 0), stop=(j == jtiles - 1),
                    )

            out_sb = constp.tile([2, C], f32, tag="out")
            nc.vector.tensor_copy(out=out_sb[:], in_=ps[:])
            nc.sync.dma_start(out=partial[:], in_=out_sb[:])

    nc.finalize()
    return nc


def _get_program(jtiles=J, jgroup=JG):
    key = (jtiles, jgroup)
    if key not in _program_cache:
        _program_cache[key] = _build_program(jtiles, jgroup)
    return _program_cache[key]


def _make_in_maps(close_er, y, max_dis, margin, ncores=NCORES, jtiles=J):
    close_er = np.ascontiguousarray(np.asarray(close_er, dtype=np.float32))
    y = np.asarray(y)
    max_dis = np.asarray(max_dis, dtype=np.float32)
    margin = np.float32(np.asarray(margin))

    npc = P * jtiles
    iota_np = np.ascontiguousarray(
        np.broadcast_to(np.arange(C, dtype=np.float16), (P, C))
    )
    dmd_np = np.ascontiguousarray(
        np.broadcast_to((max_dis + margin).astype(np.float16), (P, C))
    )
    in_maps = []
    for c in range(ncores):
        sl = slice(c * npc, (c + 1) * npc)
        in_maps.append({
            "ce": close_er[sl].reshape(P, jtiles, C).astype(np.float16),
            "ysb": np.ascontiguousarray(
                y[sl].reshape(P, jtiles).astype(np.float32)
            ),
            "iota": iota_np,
            "dmd": dmd_np,
        })
    return in_maps


def _finish(partials):
    """partials: [ncores, 2, C] -> final scalar, replicating reference math."""
    partials = np.asarray(partials, dtype=np.float64)
    sums = partials[:, 0, :].sum(axis=0)
    counts = partials[:, 1, :].sum(axis=0)
    nonempty = counts > 0
    means = np.where(nonempty, sums / np.maximum(counts, 1.0), 0.0)
    jn = nonempty.sum()
    return np.asarray(means.sum() / jn, dtype=np.float32)


def kernel(close_er, y, max_dis, margin):
    from concourse.bass_utils import run_bass_kernel_spmd

    nc = _get_program()
    in_maps = _make_in_maps(close_er, y, max_dis, margin)
    res = run_bass_kernel_spmd(nc, in_maps, list(range(NCORES)))
    partials = np.stack([res.results[i]["partial"] for i in range(NCORES)])
    return _finish(partials)


if __name__ == "__main__":
    rng = np.random.default_rng(0)
    close_er = rng.standard_normal((N, C), dtype=np.float32)
    y = rng.integers(0, C, size=N).astype(np.int32)
    max_dis = rng.standard_normal(C).astype(np.float32)
    margin = np.float32(0.5)
    out = kernel(close_er, y, max_dis, margin)
    print("kernel output:", out)
